# revision 53
# baseline (speedup 1.0000x reference)
"""Trainium2 Bass kernel for AttentionWithRoPE (B=2, S=2048, HID=2048, H=16, D=128).

Sharding (8 cores): tensor-parallel over heads x data-parallel over batch.
Core c handles batch c//4 and heads 4*(c%4) .. 4*(c%4)+4.

Numerics: projections (QKV, V, output) run as hierarchical-fp8 DoubleRow
matmuls — each operand is split on the host into hi = fp8(x*scale) and an
UNSCALED residual lo = fp8(x*scale - hi), and the product keeps the three
large cross terms (hi*hi + lo*hi + hi*lo), dropping lo*lo (~0.1% error).
DoubleRow contracts two 128-deep k-tiles per instruction at 0.5 PE
cycles/row, so the 3-term product costs 1.5 cycles per 256 contraction vs
2.0 for fp32r. The K projection keeps only 2 terms (its h-quantization
error washes out through softmax averaging). Attention (scores/exp/
colsum/PV) runs in bf16 (1 cycle/row at any tile size). End-to-end rel
err vs the fp32 reference: 1.64e-2 (budget 2e-2), deterministic.

Schedule: V projection shares phase A's hidden tiles (h loaded once);
RoPE's final add runs on the Pool/GPSIMD engine and the causal masks run
on Pool too, keeping the in-order DVE and the exp-saturated ACT off the
PE's critical path. Phase B runs a global software pipeline: score
chunks emit exp immediately, while the colsum/PV accumulation for each
chunk is deferred through a 4-deep FIFO (crossing (h,qt) iteration
boundaries), and each softmax normalization is staged by pop counts
(reciprocal first, the PE-facing broadcast 2 pops later) so no exp, mask
or reciprocal latency ever lands on the PE. Start DMAs stream on one
queue in exact consumption order (bus delivery == PE need); h-tile
prefetches ride the second hwdge queue gated by pool-slot reuse. Phase
C's first chunks interleave with the tail normalizes (per-qt attention
tiles break the false whole-tile dependency). Output partials are
written bf16 and summed on the host (the TP reduce).

Measured (TimelineSim cost model): 258843 ns, PE busy ~97% of span;
baseline fp32r kernel was 384764 ns.
"""
import numpy as np
import ml_dtypes
from contextlib import ExitStack

import concourse.bass as bass
import concourse.tile as tile
from concourse import bacc, mybir
from concourse.bass_utils import run_bass_kernel_spmd

B, S, HID = 2, 2048, 2048
H, D = 16, 128
NCORES = 8
NH = 4                 # heads per core
HC = HID // 128        # hid chunks
NP = HC // 2           # DoubleRow chunk pairs
AST = 512              # phase-A s-tile width
ANST = S // AST
QT = 512               # phase-B q-tile width
NQT = S // QT
DSCALE = float(D) ** -0.5
SH, SW, SA = 16.0, 1024.0, 16.0   # fp8 scales: hidden, weights, attn-out
F32 = mybir.dt.float32
F32R = mybir.dt.float32r
BF16 = mybir.dt.bfloat16
F8 = mybir.dt.float8e4
NF8 = ml_dtypes.float8_e4m3
NBF = ml_dtypes.bfloat16
DR = mybir.MatmulPerfMode.DoubleRow

_CACHED = {}


def _build_nc():
    nc = bacc.Bacc("TRN2", target_bir_lowering=False, debug=False,
                   num_devices=NCORES)
    hT_hi = nc.dram_tensor("hT_hi", [HID, S], F8, kind="ExternalInput")
    hT_lo = nc.dram_tensor("hT_lo", [HID, S], F8, kind="ExternalInput")
    w_in = {}
    for w in ("wq", "wk", "wv"):
        for p in ("hi", "lo"):
            w_in[f"{w}_{p}"] = nc.dram_tensor(
                f"{w}_{p}", [HID, NH * D], F8, kind="ExternalInput")
    wo_hi_d = nc.dram_tensor("wo_hi", [NH * D, HID], F8, kind="ExternalInput")
    wo_lo_d = nc.dram_tensor("wo_lo", [NH * D, HID], F8, kind="ExternalInput")
    cosT = nc.dram_tensor("cosT", [D, S], BF16, kind="ExternalInput")
    sinS = nc.dram_tensor("sinS", [D, S], BF16, kind="ExternalInput")
    tri = nc.dram_tensor("tri", [128, 128], BF16, kind="ExternalInput")
    ones = nc.dram_tensor("ones", [128, 1], BF16, kind="ExternalInput")
    onesr = nc.dram_tensor("onesr", [1, 128], F32R, kind="ExternalInput")
    out = nc.dram_tensor("out", [S, HID], BF16, kind="ExternalOutput")

    hhi_r = hT_hi.ap().rearrange("(hc p) s -> p hc s", p=128)
    hlo_r = hT_lo.ap().rearrange("(hc p) s -> p hc s", p=128)
    w_r = {k: v.ap().rearrange("(hc p) m -> p hc m", p=128)
           for k, v in w_in.items()}
    wohi_r = wo_hi_d.ap().rearrange("(g p) n -> p g n", p=128)
    wolo_r = wo_lo_d.ap().rearrange("(g p) n -> p g n", p=128)

    with tile.TileContext(nc) as tc, ExitStack() as ctx:
        constp = ctx.enter_context(tc.tile_pool(name="const", bufs=1))
        tri_sb = constp.tile([128, 128], BF16, tag="tri", name="tri")
        ones_sb = constp.tile([128, 1], BF16, tag="ones", name="ones")
        onesr_sb = constp.tile([1, 128], F32R, tag="onesr", name="onesr")

        wop = ctx.enter_context(tc.tile_pool(name="wo", bufs=1))
        wo_hi = wop.tile([128, NH, HID], F8, tag="wohi", name="wohi")
        wo_lo = wop.tile([128, NH, HID], F8, tag="wolo", name="wolo")

        # Q^T/K^T (bf16) resident through attention; V natural orientation.
        # One tile per s-tile so phase-B reads only depend on the s-tile
        # that produced them (Tile tracks deps at whole-tile granularity).
        qkp = ctx.enter_context(tc.tile_pool(name="qk", bufs=1))
        qsb = [qkp.tile([128, NH, AST], BF16, tag=f"qsb{t}", name=f"qsb{t}")
               for t in range(ANST)]
        ksb = [qkp.tile([128, NH, AST], BF16, tag=f"ksb{t}", name=f"ksb{t}")
               for t in range(ANST)]
        v_sb = [qkp.tile([128, AST // 128, NH * D], BF16, tag=f"vsb{t}",
                         name=f"vsb{t}") for t in range(ANST)]
        at_hi = [qkp.tile([128, NH, QT], F8, tag=f"athi{t}",
                          name=f"athi{t}") for t in range(NQT)]
        at_lo = [qkp.tile([128, NH, QT], F8, tag=f"atlo{t}",
                          name=f"atlo{t}") for t in range(NQT)]

        # phase-A-scoped pools (weights + h tiles free ~14MB before B/C)
        wstack = ExitStack()
        wpool = wstack.enter_context(tc.tile_pool(name="w", bufs=1))
        w_sb = {k: wpool.tile([128, HC, NH * D], F8, tag=k, name=k)
                for k in w_in}
        hpool = wstack.enter_context(tc.tile_pool(name="ah", bufs=3))
        cspool = wstack.enter_context(tc.tile_pool(name="acs", bufs=2))

        def load_htile(st):
            # prefetch path (st>=1): single DMAs on the ACT hwdge queue
            sl = bass.ts(st, AST)
            hb_hi = hpool.tile([128, HC, AST], F8, tag="hhi", name="hhi")
            hb_lo = hpool.tile([128, HC, AST], F8, tag="hlo", name="hlo")
            nc.scalar.dma_start(hb_hi[:], hhi_r[:, :, sl])
            cs_t = cspool.tile([128, AST], BF16, tag="cs", name="cs")
            nc.scalar.dma_start(cs_t[:], cosT.ap()[:, sl])
            ss_t = cspool.tile([128, AST], BF16, tag="ss", name="ss")
            nc.scalar.dma_start(ss_t[:], sinS.ap()[:, sl])
            nc.scalar.dma_start(hb_lo[:], hlo_r[:, :, sl])
            return hb_hi, hb_lo, cs_t, ss_t

        # Start-critical stream on SP: wq_hi and the first h tile, chunk-
        # interleaved so T1 matmuls start on the first chunk pair. The
        # rest rides the ACT hwdge queue in parallel.
        # Single SP stream in strict consumption order so the shared DMA
        # bus delivers bytes exactly as the PE needs them. Later tiles ride
        # the ACT queue, naturally gated by h-pool slot reuse.
        sl0 = bass.ts(0, AST)
        sl1 = bass.ts(1, AST)
        hb_hi0 = hpool.tile([128, HC, AST], F8, tag="hhi", name="hhi")
        hb_lo0 = hpool.tile([128, HC, AST], F8, tag="hlo", name="hlo")
        # strict consumption-order on one queue: bus delivery == PE need
        for c in range(4):
            h4 = slice(4 * c, 4 * c + 4)
            nc.sync.dma_start(w_sb["wq_hi"][:, h4, :], w_r["wq_hi"][:, h4, :])
            nc.sync.dma_start(hb_hi0[:, h4, :], hhi_r[:, h4, sl0])
        for hh in (slice(0, 8), slice(8, 16)):
            nc.sync.dma_start(w_sb["wq_lo"][:, hh, :], w_r["wq_lo"][:, hh, :])
            nc.sync.dma_start(hb_lo0[:, hh, :], hlo_r[:, hh, sl0])
        cs_t0 = cspool.tile([128, AST], BF16, tag="cs", name="cs")
        nc.sync.dma_start(cs_t0[:], cosT.ap()[:, sl0])
        ss_t0 = cspool.tile([128, AST], BF16, tag="ss", name="ss")
        nc.sync.dma_start(ss_t0[:], sinS.ap()[:, sl0])
        nc.sync.dma_start(w_sb["wk_hi"][:], w_r["wk_hi"][:, :, :])
        nc.sync.dma_start(w_sb["wk_lo"][:], w_r["wk_lo"][:, :, :])
        nc.sync.dma_start(w_sb["wv_hi"][:], w_r["wv_hi"][:, :, :])
        nc.sync.dma_start(w_sb["wv_lo"][:], w_r["wv_lo"][:, :, :])
        hb_hi1 = hpool.tile([128, HC, AST], F8, tag="hhi", name="hhi")
        hb_lo1 = hpool.tile([128, HC, AST], F8, tag="hlo", name="hlo")
        nc.sync.dma_start(hb_hi1[:], hhi_r[:, :, sl1])
        nc.sync.dma_start(hb_lo1[:], hlo_r[:, :, sl1])
        cs_t1 = cspool.tile([128, AST], BF16, tag="cs", name="cs")
        nc.sync.dma_start(cs_t1[:], cosT.ap()[:, sl1])
        ss_t1 = cspool.tile([128, AST], BF16, tag="ss", name="ss")
        nc.sync.dma_start(ss_t1[:], sinS.ap()[:, sl1])
        nc.sync.dma_start(tri_sb[:], tri.ap())
        nc.sync.dma_start(ones_sb[:], ones.ap())
        nc.sync.dma_start(onesr_sb[:], onesr.ap())
        nc.sync.dma_start(wo_hi[:], wohi_r[:, :, :])
        nc.sync.dma_start(wo_lo[:], wolo_r[:, :, :])
        htiles = {0: (hb_hi0, hb_lo0, cs_t0, ss_t0),
                  1: (hb_hi1, hb_lo1, cs_t1, ss_t1)}

        # ================= Phase A: QKV projections + RoPE ============
        with ExitStack() as astack:
            ropep = astack.enter_context(tc.tile_pool(name="arope", bufs=2))
            psA = astack.enter_context(
                tc.tile_pool(name="apsqk", bufs=8, space="PSUM"))

            for st in range(ANST):
                hb_hi, hb_lo, cs_t, ss_t = (htiles[st] if st in htiles
                                            else load_htile(st))
                def v_pass():
                    for sc in range(AST // 128):
                        ssl = slice(sc * 128, (sc + 1) * 128)
                        ps = psA.tile([128, NH * D], F32, tag="psqk",
                                      name="psv")
                        for j in range(NP):
                            jp = slice(2 * j, 2 * j + 2)
                            nc.tensor.matmul(
                                ps[:], hb_hi[:, jp, ssl],
                                w_sb["wv_hi"][:, jp, :],
                                start=(j == 0), stop=False, perf_mode=DR,
                                skip_group_check=True)
                        for j in range(NP):
                            jp = slice(2 * j, 2 * j + 2)
                            nc.tensor.matmul(
                                ps[:], hb_lo[:, jp, ssl],
                                w_sb["wv_hi"][:, jp, :],
                                start=False, stop=False, perf_mode=DR,
                                skip_group_check=True)
                        for j in range(NP):
                            jp = slice(2 * j, 2 * j + 2)
                            nc.tensor.matmul(
                                ps[:], hb_hi[:, jp, ssl],
                                w_sb["wv_lo"][:, jp, :],
                                start=False, stop=(j == NP - 1), perf_mode=DR,
                                skip_group_check=True)
                        with nc.allow_low_precision(reason="bf16 v"):
                            if st == ANST - 1 and sc % 2 == 1:
                                # split the last tile's v-copies across ACT
                                # and DVE: halves the drain that gates the
                                # phase-B pool-open rendezvous
                                nc.vector.tensor_scalar(
                                    v_sb[st][:, sc, :], ps[:],
                                    1.0 / (SH * SW), None,
                                    mybir.AluOpType.mult)
                            else:
                                nc.scalar.mul(v_sb[st][:, sc, :],
                                              ps[:], 1.0 / (SH * SW))

                for pi, (whi, wlo, dsb) in enumerate((
                        (w_sb["wq_hi"], w_sb["wq_lo"], qsb[st]),
                        (w_sb["wk_hi"], w_sb["wk_lo"], ksb[st]))):
                    # j-outer/h-inner: each chunk pair is consumed by all
                    # heads as soon as its DMA lands (start-latency path)
                    pss = [psA.tile([128, AST], F32, tag="psqk", name="psqk")
                           for _ in range(NH)]
                    # K drops the w_hi*h_lo term: its 3.5% h-quantization
                    # error washes out through softmax averaging (measured
                    # 1.4e-2 end-to-end vs the 2e-2 budget) and saves 14us
                    # of PE time. Q and V keep all three terms.
                    terms = [(whi, hb_hi), (wlo, hb_hi)]
                    if pi == 0:
                        terms.append((whi, hb_lo))
                    for ti, (wt, hb) in enumerate(terms):
                        for j in range(NP):
                            jp = slice(2 * j, 2 * j + 2)
                            for h in range(NH):
                                hD = slice(h * D, (h + 1) * D)
                                nc.tensor.matmul(
                                    pss[h][:], wt[:, jp, hD], hb[:, jp, :],
                                    start=(ti == 0 and j == 0),
                                    stop=(ti == len(terms) - 1
                                          and j == NP - 1),
                                    perf_mode=DR, skip_group_check=True)
                    for h in range(NH):
                        # RoPE fused on DVE reading projection PSUM
                        # (cos/sin arrive pre-scaled by 1/(SH*SW)).
                        ps = pss[h]
                        tsin = ropep.tile([128, AST], F32, tag="tsin",
                                          name="tsin")
                        nc.vector.tensor_tensor(
                            tsin[0:64, :], ps[64:128, :], ss_t[0:64, :],
                            mybir.AluOpType.mult)
                        nc.vector.tensor_tensor(
                            tsin[64:128, :], ps[0:64, :], ss_t[64:128, :],
                            mybir.AluOpType.mult)
                        tcos = ropep.tile([128, AST], F32, tag="tcos",
                                          name="tcos")
                        nc.vector.tensor_tensor(
                            tcos[:], ps[:], cs_t[:], mybir.AluOpType.mult)
                        # final add on the Pool/GPSIMD engine (third
                        # elementwise lane; keeps DVE from lagging the PE)
                        with nc.allow_low_precision(reason="bf16 q/k"):
                            nc.gpsimd.tensor_tensor(
                                dsb[:, h, :], tcos[:], tsin[:],
                                mybir.AluOpType.add)
                v_pass()


        wstack.close()

        # ================= Phase B: attention =================
        with ExitStack() as bctx:
            smallp = bctx.enter_context(tc.tile_pool(name="bsmall", bufs=3))
            psPV = bctx.enter_context(
                tc.tile_pool(name="bpspv", bufs=2, space="PSUM"))
            psCS = bctx.enter_context(
                tc.tile_pool(name="bpscs", bufs=1, space="PSUM"))
            psRB = bctx.enter_context(
                tc.tile_pool(name="bpsrb", bufs=1, space="PSUM"))
            sstack = ExitStack()
            expp = sstack.enter_context(tc.tile_pool(name="bexp", bufs=7))
            psS = sstack.enter_context(
                tc.tile_pool(name="bpss", bufs=3, space="PSUM"))

            def emit_recip(pend):
                # DVE part only — emitted early so the reciprocal clears
                # the DVE queue before the PE reaches the rbc matmul
                h, qt, pvps, csps = pend
                rec = smallp.tile([1, QT], F32R, tag="rec", name="rec")
                with nc.allow_low_precision(
                        reason="softmax denom reciprocal to f32r"):
                    nc.vector.reciprocal(rec[:], csps[:])
                return rec

            def emit_norm_rest(pend, rec):
                h, qt, pvps, csps = pend
                # broadcast SA/den to 128 partitions via K=1 PE matmul
                rbc = psRB.tile([128, QT], F32, tag="rbc", name="rbc")
                nc.tensor.matmul(rbc[:], onesr_sb[:], rec[:],
                                 start=True, stop=True)
                # ACT is saturated by exp in phase B — keep these copies
                # on DVE/Pool instead
                at_t = smallp.tile([128, QT], F32, tag="att", name="att")
                nc.vector.tensor_copy(at_t[:], pvps[:])
                tmp = smallp.tile([128, QT], F32, tag="tmp", name="tmp")
                nc.vector.tensor_tensor(tmp[:], at_t[:], rbc[:],
                                        mybir.AluOpType.mult)
                with nc.allow_low_precision(reason="fp8 attn split"):
                    nc.vector.tensor_copy(at_hi[qt][:, h, :], tmp[:])
                    nc.vector.tensor_tensor(
                        at_lo[qt][:, h, :], tmp[:],
                        at_hi[qt][:, h, :], mybir.AluOpType.subtract)

            from collections import deque
            cspv_q = deque()     # (emit_fn, key_or_None_if_not_last)
            norms = []           # dicts: pend, rec, recip_at, rest_at
            popc = [0]
            LAG = 4
            TOTAL_POPS = NH * sum((QT // 128) * (qt + 1) for qt in range(NQT))

            def pop_one():
                fn, last_key = cspv_q.popleft()
                fn()
                popc[0] += 1
                if last_key is not None:
                    norms.append({"pend": last_key,
                                  "recip_at": popc[0] + 1,
                                  "rest_at": popc[0] + 3, "rec": None})
                for nrm in norms:
                    if nrm["rec"] is None and popc[0] >= nrm["recip_at"]:
                        nrm["rec"] = emit_recip(nrm["pend"])
                for nrm in norms[:]:
                    # rests landing at the tail would serialize against the
                    # drain; defer them past phase C's first chunks instead
                    if (nrm["rec"] is not None and popc[0] >= nrm["rest_at"]
                            and nrm["rest_at"] <= TOTAL_POPS - 2):
                        emit_norm_rest(nrm["pend"], nrm["rec"])
                        norms.remove(nrm)

            for h in range(NH):
                for qt in range(NQT):
                    nallow = (QT // 128) * qt + (QT // 128)
                    pvps = psPV.tile([128, QT], F32, tag="pv", name="pv")
                    csps = psCS.tile([1, QT], F32, tag="cs", name="cs")

                    def mk_cspv(i, kc, lo, eb, pvps=pvps, csps=csps,
                                h=h, nallow=nallow):
                        def emit():
                            nc.tensor.matmul(
                                csps[:, lo:QT], ones_sb[:], eb[:, lo:QT],
                                start=(i == 0), stop=(i == nallow - 1),
                                skip_group_check=True)
                            nc.tensor.matmul(
                                pvps[:, lo:QT],
                                v_sb[kc // 4][:, kc % 4,
                                              h * D:(h + 1) * D],
                                eb[:, lo:QT],
                                start=(i == 0), stop=(i == nallow - 1),
                                skip_group_check=True)
                        return emit

                    for kc in range(nallow):
                        j = kc - (QT // 128) * qt
                        lo = max(0, 128 * j)
                        sps = psS.tile([128, QT], F32, tag="s", name="s")
                        nc.tensor.matmul(
                            sps[:, lo:QT],
                            ksb[kc // 4][:, h, (kc % 4) * 128:
                                         (kc % 4 + 1) * 128],
                            qsb[qt][:, h, lo:QT],
                            start=True, stop=True)
                        eb = expp.tile([128, QT], BF16, tag="e", name="e")
                        nc.scalar.activation(
                            eb[:, lo:QT], sps[:, lo:QT],
                            mybir.ActivationFunctionType.Exp, scale=DSCALE)
                        if j >= 0:
                            # mask on Pool: DVE is in-order and the
                            # normalize recip/tmp would delay it
                            with nc.allow_low_precision(reason="bf16 mask"):
                                nc.gpsimd.tensor_tensor(
                                    eb[:, lo:lo + 128],
                                    eb[:, lo:lo + 128].bitcast(BF16),
                                    tri_sb[:], mybir.AluOpType.mult)
                        cspv_q.append(
                            (mk_cspv(kc, kc, lo, eb),
                             (h, qt, pvps, csps) if kc == nallow - 1
                             else None))
                        if len(cspv_q) > LAG:
                            pop_one()
            while cspv_q:
                pop_one()
            for nrm in norms:
                if nrm["rec"] is None:
                    nrm["rec"] = emit_recip(nrm["pend"])

            # ============ Phase C: output projection ============
            # psS's banks are free (closed); psO shares the PSUM budget
            # with the still-live psPV/psCS/psRB so phase C's first chunks
            # can interleave with the deferred tail normalizes.
            sstack.close()
            outp = bctx.enter_context(tc.tile_pool(name="cout", bufs=2))
            psO = bctx.enter_context(
                tc.tile_pool(name="cpso", bufs=4, space="PSUM"))

            def emit_c_chunk(sc):
                ssl = bass.ts(sc, 128)
                ot = outp.tile([128, HID], BF16, tag="ot", name="ot")
                for nt in range(HID // QT):
                    nsl = bass.ts(nt, QT)
                    ps = psO.tile([128, QT], F32, tag="o", name="o")
                    ati = sc // 4
                    asl = slice((sc % 4) * 128, (sc % 4 + 1) * 128)
                    for gp in range(NH // 2):
                        g2 = slice(2 * gp, 2 * gp + 2)
                        nc.tensor.matmul(
                            ps[:], at_hi[ati][:, g2, asl], wo_hi[:, g2, nsl],
                            start=(gp == 0), stop=False, perf_mode=DR,
                            skip_group_check=True)
                    for gp in range(NH // 2):
                        g2 = slice(2 * gp, 2 * gp + 2)
                        nc.tensor.matmul(
                            ps[:], at_lo[ati][:, g2, asl], wo_hi[:, g2, nsl],
                            start=False, stop=False, perf_mode=DR,
                            skip_group_check=True)
                    for gp in range(NH // 2):
                        g2 = slice(2 * gp, 2 * gp + 2)
                        nc.tensor.matmul(
                            ps[:], at_hi[ati][:, g2, asl], wo_lo[:, g2, nsl],
                            start=False, stop=(gp == NH // 2 - 1),
                            perf_mode=DR, skip_group_check=True)
                    with nc.allow_low_precision(reason="bf16 out"):
                        if nt % 2 == 0:
                            nc.vector.tensor_scalar(
                                ot[:, nsl], ps[:], 1.0 / (SA * SW), None,
                                mybir.AluOpType.mult)
                        else:
                            nc.scalar.mul(ot[:, nsl], ps[:], 1.0 / (SA * SW))
                    if sc == S // 128 - 1:
                        # alternate hwdge queues so the tail dispatches overlap
                        eng = nc.sync if nt % 2 == 0 else nc.scalar
                        eng.dma_start(out.ap()[ssl, nsl], ot[:, nsl])
                    elif nt == 1:
                        nc.sync.dma_start(out.ap()[ssl, 0:2 * QT],
                                          ot[:, 0:2 * QT])
                if sc != S // 128 - 1:
                    nc.sync.dma_start(out.ap()[ssl, 2 * QT:HID],
                                      ot[:, 2 * QT:HID])

            emit_c_chunk(0)
            emit_c_chunk(1)
            for nrm in norms[:]:
                emit_norm_rest(nrm["pend"], nrm["rec"])
                norms.remove(nrm)
            for sc in range(2, S // 128):
                emit_c_chunk(sc)

    nc.compile()
    return nc


def _split8(x, scale):
    xs = x * scale
    hi = np.asarray(xs, dtype=NF8)
    lo = np.asarray(xs - hi.astype(np.float32), dtype=NF8)
    return hi, lo


def _prep_in_maps(hidden_states, cos, sin, w_qkv, w_o):
    hs = np.asarray(hidden_states, dtype=np.float32)
    cos = np.asarray(cos, dtype=np.float32)
    sin = np.asarray(sin, dtype=np.float32)
    w_qkv = np.asarray(w_qkv, dtype=np.float32)
    w_o = np.asarray(w_o, dtype=np.float32)

    wT = np.ascontiguousarray(w_qkv.T)          # (HID, 3*H*D)
    woTf = np.ascontiguousarray(w_o.T)          # (H*D, HID)
    cosT = np.asarray(np.ascontiguousarray(cos.T) / (SH * SW), dtype=NBF)
    sinT = np.ascontiguousarray(sin.T) / (SH * SW)
    sinS = sinT.copy()
    sinS[:64] = -sinT[:64]
    sinS = np.asarray(sinS, dtype=NBF)
    tri = np.asarray(np.triu(np.ones((128, 128), np.float32)), dtype=NBF)
    ones = np.asarray(np.ones((128, 1), np.float32), dtype=NBF)
    onesr = np.full((1, 128), SA, np.float32)

    hT = [np.ascontiguousarray(hs[b].T) for b in range(B)]
    hT8 = [_split8(h, SH) for h in hT]

    in_maps = []
    for c in range(NCORES):
        b, hg = c // 4, c % 4
        lo, hi = hg * NH * D, (hg + 1) * NH * D
        wq_hi, wq_lo = _split8(np.ascontiguousarray(wT[:, lo:hi]), SW)
        wk_hi, wk_lo = _split8(
            np.ascontiguousarray(wT[:, H * D + lo:H * D + hi]), SW)
        wv_hi, wv_lo = _split8(
            np.ascontiguousarray(wT[:, 2 * H * D + lo:2 * H * D + hi]), SW)
        wo_hi, wo_lo = _split8(np.ascontiguousarray(woTf[lo:hi, :]), SW)
        in_maps.append({
            "hT_hi": hT8[b][0], "hT_lo": hT8[b][1],
            "wq_hi": wq_hi, "wq_lo": wq_lo,
            "wk_hi": wk_hi, "wk_lo": wk_lo,
            "wv_hi": wv_hi, "wv_lo": wv_lo,
            "wo_hi": wo_hi, "wo_lo": wo_lo,
            "cosT": cosT, "sinS": sinS,
            "tri": tri, "ones": ones, "onesr": onesr,
        })
    return in_maps


def kernel(hidden_states, cos, sin, w_qkv, w_o, _trace=False):
    if "nc" not in _CACHED:
        _CACHED["nc"] = _build_nc()
    nc = _CACHED["nc"]
    in_maps = _prep_in_maps(hidden_states, cos, sin, w_qkv, w_o)
    res = run_bass_kernel_spmd(nc, in_maps, core_ids=list(range(NCORES)),
                               trace=_trace)
    _CACHED["last_result"] = res
    out = np.zeros((B, S, HID), np.float32)
    for c in range(NCORES):
        out[c // 4] += res.results[c]["out"].astype(np.float32)
    return out


# revision 54
# speedup vs baseline: 1.0149x; 1.0149x over previous
"""Trainium2 Bass kernel for AttentionWithRoPE (B=2, S=2048, HID=2048, H=16, D=128).

Sharding (8 cores): tensor-parallel over heads x data-parallel over batch.
Core c handles batch c//4 and heads 4*(c%4) .. 4*(c%4)+4.

Numerics: projections (QKV, V, output) run as hierarchical-fp8 DoubleRow
matmuls — each operand is split on the host into hi = fp8(x*scale) and an
UNSCALED residual lo = fp8(x*scale - hi), and the product keeps the three
large cross terms (hi*hi + lo*hi + hi*lo), dropping lo*lo (~0.1% error).
DoubleRow contracts two 128-deep k-tiles per instruction at 0.5 PE
cycles/row, so the 3-term product costs 1.5 cycles per 256 contraction vs
2.0 for fp32r. The K projection keeps only 2 terms (its h-quantization
error washes out through softmax averaging). Attention (scores/exp/
colsum/PV) runs in bf16 (1 cycle/row at any tile size). End-to-end rel
err vs the fp32 reference: 1.64e-2 (budget 2e-2), deterministic.

Schedule: V projection shares phase A's hidden tiles (h loaded once);
RoPE's final add runs on the Pool/GPSIMD engine and the causal masks run
on Pool too, keeping the in-order DVE and the exp-saturated ACT off the
PE's critical path. Phase B runs a global software pipeline: score
chunks emit exp immediately, while the colsum/PV accumulation for each
chunk is deferred through a 4-deep FIFO (crossing (h,qt) iteration
boundaries), and each softmax normalization is staged by pop counts
(reciprocal first, the PE-facing broadcast 2 pops later) so no exp, mask
or reciprocal latency ever lands on the PE. Start DMAs stream on one
queue in exact consumption order (bus delivery == PE need); h-tile
prefetches ride the second hwdge queue gated by pool-slot reuse. Phase
C's first chunks interleave with the tail normalizes (per-qt attention
tiles break the false whole-tile dependency). Output partials are
written bf16 and summed on the host (the TP reduce).

Measured (TimelineSim cost model): 258843 ns, PE busy ~97% of span;
baseline fp32r kernel was 384764 ns.
"""
import numpy as np
import ml_dtypes
from contextlib import ExitStack

import concourse.bass as bass
import concourse.tile as tile
from concourse import bacc, mybir
from concourse.bass_utils import run_bass_kernel_spmd

B, S, HID = 2, 2048, 2048
H, D = 16, 128
NCORES = 8
NH = 4                 # heads per core
HC = HID // 128        # hid chunks
NP = HC // 2           # DoubleRow chunk pairs
AST = 512              # phase-A s-tile width
ANST = S // AST
QT = 512               # phase-B q-tile width
NQT = S // QT
DSCALE = float(D) ** -0.5
SH, SW, SA = 16.0, 1024.0, 16.0   # fp8 scales: hidden, weights, attn-out
F32 = mybir.dt.float32
F32R = mybir.dt.float32r
BF16 = mybir.dt.bfloat16
F8 = mybir.dt.float8e4
NF8 = ml_dtypes.float8_e4m3
NBF = ml_dtypes.bfloat16
DR = mybir.MatmulPerfMode.DoubleRow

_CACHED = {}


def _build_nc():
    nc = bacc.Bacc("TRN2", target_bir_lowering=False, debug=False,
                   num_devices=NCORES)
    hT_hi = nc.dram_tensor("hT_hi", [HID, S], F8, kind="ExternalInput")
    hT_lo = nc.dram_tensor("hT_lo", [HID, S], F8, kind="ExternalInput")
    w_in = {}
    for w in ("wq", "wk", "wv"):
        for p in ("hi", "lo"):
            w_in[f"{w}_{p}"] = nc.dram_tensor(
                f"{w}_{p}", [HID, NH * D], F8, kind="ExternalInput")
    wo_hi_d = nc.dram_tensor("wo_hi", [NH * D, HID], F8, kind="ExternalInput")
    wo_lo_d = nc.dram_tensor("wo_lo", [NH * D, HID], F8, kind="ExternalInput")
    cosT = nc.dram_tensor("cosT", [D, S], BF16, kind="ExternalInput")
    sinS = nc.dram_tensor("sinS", [D, S], BF16, kind="ExternalInput")
    tri = nc.dram_tensor("tri", [128, 128], BF16, kind="ExternalInput")
    ones = nc.dram_tensor("ones", [128, 1], BF16, kind="ExternalInput")
    onesr = nc.dram_tensor("onesr", [1, 128], F32R, kind="ExternalInput")
    out = nc.dram_tensor("out", [S, HID], BF16, kind="ExternalOutput")

    hhi_r = hT_hi.ap().rearrange("(hc p) s -> p hc s", p=128)
    hlo_r = hT_lo.ap().rearrange("(hc p) s -> p hc s", p=128)
    w_r = {k: v.ap().rearrange("(hc p) m -> p hc m", p=128)
           for k, v in w_in.items()}
    wohi_r = wo_hi_d.ap().rearrange("(g p) n -> p g n", p=128)
    wolo_r = wo_lo_d.ap().rearrange("(g p) n -> p g n", p=128)

    with tile.TileContext(nc) as tc, ExitStack() as ctx:
        constp = ctx.enter_context(tc.tile_pool(name="const", bufs=1))
        tri_sb = constp.tile([128, 128], BF16, tag="tri", name="tri")
        ones_sb = constp.tile([128, 1], BF16, tag="ones", name="ones")
        onesr_sb = constp.tile([1, 128], F32R, tag="onesr", name="onesr")

        wop = ctx.enter_context(tc.tile_pool(name="wo", bufs=1))
        wo_hi = wop.tile([128, NH, HID], F8, tag="wohi", name="wohi")
        wo_lo = wop.tile([128, NH, HID], F8, tag="wolo", name="wolo")

        # Q^T/K^T (bf16) resident through attention; V natural orientation.
        # One tile per s-tile so phase-B reads only depend on the s-tile
        # that produced them (Tile tracks deps at whole-tile granularity).
        qkp = ctx.enter_context(tc.tile_pool(name="qk", bufs=1))
        qsb = [qkp.tile([128, NH, AST], BF16, tag=f"qsb{t}", name=f"qsb{t}")
               for t in range(ANST)]
        ksb = [qkp.tile([128, NH, AST], BF16, tag=f"ksb{t}", name=f"ksb{t}")
               for t in range(ANST)]
        v_sb = [qkp.tile([128, AST // 128, NH * D], BF16, tag=f"vsb{t}",
                         name=f"vsb{t}") for t in range(ANST)]
        at_hi = [qkp.tile([128, NH, QT], F8, tag=f"athi{t}",
                          name=f"athi{t}") for t in range(NQT)]
        at_lo = [qkp.tile([128, NH, QT], F8, tag=f"atlo{t}",
                          name=f"atlo{t}") for t in range(NQT)]

        # phase-A-scoped pools (weights + h tiles free ~14MB before B/C)
        wstack = ExitStack()
        wpool = wstack.enter_context(tc.tile_pool(name="w", bufs=1))
        w_sb = {k: wpool.tile([128, HC, NH * D], F8, tag=k, name=k)
                for k in w_in}
        hpool = wstack.enter_context(tc.tile_pool(name="ah", bufs=3))
        cspool = wstack.enter_context(tc.tile_pool(name="acs", bufs=2))

        def load_htile(st):
            # prefetch path (st>=1): single DMAs on the ACT hwdge queue
            sl = bass.ts(st, AST)
            hb_hi = hpool.tile([128, HC, AST], F8, tag="hhi", name="hhi")
            hb_lo = hpool.tile([128, HC, AST], F8, tag="hlo", name="hlo")
            nc.scalar.dma_start(hb_hi[:], hhi_r[:, :, sl])
            cs_t = cspool.tile([128, AST], BF16, tag="cs", name="cs")
            nc.scalar.dma_start(cs_t[:], cosT.ap()[:, sl])
            ss_t = cspool.tile([128, AST], BF16, tag="ss", name="ss")
            nc.scalar.dma_start(ss_t[:], sinS.ap()[:, sl])
            nc.scalar.dma_start(hb_lo[:], hlo_r[:, :, sl])
            return hb_hi, hb_lo, cs_t, ss_t

        # Start-critical stream on SP: wq_hi and the first h tile, chunk-
        # interleaved so T1 matmuls start on the first chunk pair. The
        # rest rides the ACT hwdge queue in parallel.
        # Single SP stream in strict consumption order so the shared DMA
        # bus delivers bytes exactly as the PE needs them. Later tiles ride
        # the ACT queue, naturally gated by h-pool slot reuse.
        sl0 = bass.ts(0, AST)
        sl1 = bass.ts(1, AST)
        hb_hi0 = hpool.tile([128, HC, AST], F8, tag="hhi", name="hhi")
        hb_lo0 = hpool.tile([128, HC, AST], F8, tag="hlo", name="hlo")
        # strict consumption-order on one queue: bus delivery == PE need
        for c in range(4):
            h4 = slice(4 * c, 4 * c + 4)
            nc.sync.dma_start(w_sb["wq_hi"][:, h4, :], w_r["wq_hi"][:, h4, :])
            nc.sync.dma_start(hb_hi0[:, h4, :], hhi_r[:, h4, sl0])
        for hh in (slice(0, 8), slice(8, 16)):
            nc.sync.dma_start(w_sb["wq_lo"][:, hh, :], w_r["wq_lo"][:, hh, :])
            nc.sync.dma_start(hb_lo0[:, hh, :], hlo_r[:, hh, sl0])
        cs_t0 = cspool.tile([128, AST], BF16, tag="cs", name="cs")
        nc.sync.dma_start(cs_t0[:], cosT.ap()[:, sl0])
        ss_t0 = cspool.tile([128, AST], BF16, tag="ss", name="ss")
        nc.sync.dma_start(ss_t0[:], sinS.ap()[:, sl0])
        nc.sync.dma_start(w_sb["wk_hi"][:], w_r["wk_hi"][:, :, :])
        nc.sync.dma_start(w_sb["wk_lo"][:], w_r["wk_lo"][:, :, :])
        nc.sync.dma_start(w_sb["wv_hi"][:], w_r["wv_hi"][:, :, :])
        nc.sync.dma_start(w_sb["wv_lo"][:], w_r["wv_lo"][:, :, :])
        hb_hi1 = hpool.tile([128, HC, AST], F8, tag="hhi", name="hhi")
        hb_lo1 = hpool.tile([128, HC, AST], F8, tag="hlo", name="hlo")
        nc.sync.dma_start(hb_hi1[:], hhi_r[:, :, sl1])
        nc.sync.dma_start(hb_lo1[:], hlo_r[:, :, sl1])
        cs_t1 = cspool.tile([128, AST], BF16, tag="cs", name="cs")
        nc.sync.dma_start(cs_t1[:], cosT.ap()[:, sl1])
        ss_t1 = cspool.tile([128, AST], BF16, tag="ss", name="ss")
        nc.sync.dma_start(ss_t1[:], sinS.ap()[:, sl1])
        nc.sync.dma_start(tri_sb[:], tri.ap())
        nc.sync.dma_start(ones_sb[:], ones.ap())
        nc.sync.dma_start(onesr_sb[:], onesr.ap())
        nc.sync.dma_start(wo_hi[:], wohi_r[:, :, :])
        nc.sync.dma_start(wo_lo[:], wolo_r[:, :, :])
        htiles = {0: (hb_hi0, hb_lo0, cs_t0, ss_t0),
                  1: (hb_hi1, hb_lo1, cs_t1, ss_t1)}

        # ================= Phase A: QKV projections + RoPE ============
        with ExitStack() as astack:
            ropep = astack.enter_context(tc.tile_pool(name="arope", bufs=2))
            psA = astack.enter_context(
                tc.tile_pool(name="apsqk", bufs=8, space="PSUM"))

            for st in range(ANST):
                hb_hi, hb_lo, cs_t, ss_t = (htiles[st] if st in htiles
                                            else load_htile(st))
                def v_pass():
                    for sc in range(AST // 128):
                        ssl = slice(sc * 128, (sc + 1) * 128)
                        ps = psA.tile([128, NH * D], F32, tag="psqk",
                                      name="psv")
                        for j in range(NP):
                            jp = slice(2 * j, 2 * j + 2)
                            nc.tensor.matmul(
                                ps[:], hb_hi[:, jp, ssl],
                                w_sb["wv_hi"][:, jp, :],
                                start=(j == 0), stop=False, perf_mode=DR,
                                skip_group_check=True)
                        for j in range(NP):
                            jp = slice(2 * j, 2 * j + 2)
                            nc.tensor.matmul(
                                ps[:], hb_lo[:, jp, ssl],
                                w_sb["wv_hi"][:, jp, :],
                                start=False, stop=False, perf_mode=DR,
                                skip_group_check=True)
                        for j in range(NP):
                            jp = slice(2 * j, 2 * j + 2)
                            nc.tensor.matmul(
                                ps[:], hb_hi[:, jp, ssl],
                                w_sb["wv_lo"][:, jp, :],
                                start=False, stop=(j == NP - 1), perf_mode=DR,
                                skip_group_check=True)
                        with nc.allow_low_precision(reason="bf16 v"):
                            nc.scalar.mul(v_sb[st][:, sc, :],
                                          ps[:], 1.0 / (SH * SW))

                for pi, (whi, wlo, dsb) in enumerate((
                        (w_sb["wq_hi"], w_sb["wq_lo"], qsb[st]),
                        (w_sb["wk_hi"], w_sb["wk_lo"], ksb[st]))):
                    # j-outer/h-inner: each chunk pair is consumed by all
                    # heads as soon as its DMA lands (start-latency path)
                    pss = [psA.tile([128, AST], F32, tag="psqk", name="psqk")
                           for _ in range(NH)]
                    # K drops the w_hi*h_lo term: its 3.5% h-quantization
                    # error washes out through softmax averaging (measured
                    # 1.4e-2 end-to-end vs the 2e-2 budget) and saves 14us
                    # of PE time. Q and V keep all three terms.
                    terms = [(whi, hb_hi), (wlo, hb_hi)]
                    if pi == 0:
                        terms.append((whi, hb_lo))
                    for ti, (wt, hb) in enumerate(terms):
                        for j in range(NP):
                            jp = slice(2 * j, 2 * j + 2)
                            for h in range(NH):
                                hD = slice(h * D, (h + 1) * D)
                                nc.tensor.matmul(
                                    pss[h][:], wt[:, jp, hD], hb[:, jp, :],
                                    start=(ti == 0 and j == 0),
                                    stop=(ti == len(terms) - 1
                                          and j == NP - 1),
                                    perf_mode=DR, skip_group_check=True)
                    for h in range(NH):
                        # RoPE fused on DVE reading projection PSUM
                        # (cos/sin arrive pre-scaled by 1/(SH*SW)).
                        ps = pss[h]
                        tsin = ropep.tile([128, AST], F32, tag="tsin",
                                          name="tsin")
                        nc.vector.tensor_tensor(
                            tsin[0:64, :], ps[64:128, :], ss_t[0:64, :],
                            mybir.AluOpType.mult)
                        nc.vector.tensor_tensor(
                            tsin[64:128, :], ps[0:64, :], ss_t[64:128, :],
                            mybir.AluOpType.mult)
                        tcos = ropep.tile([128, AST], F32, tag="tcos",
                                          name="tcos")
                        nc.vector.tensor_tensor(
                            tcos[:], ps[:], cs_t[:], mybir.AluOpType.mult)
                        # final add on the Pool/GPSIMD engine (third
                        # elementwise lane; keeps DVE from lagging the PE)
                        with nc.allow_low_precision(reason="bf16 q/k"):
                            nc.gpsimd.tensor_tensor(
                                dsb[:, h, :], tcos[:], tsin[:],
                                mybir.AluOpType.add)
                v_pass()


        wstack.close()

        # ================= Phase B: attention =================
        with ExitStack() as bctx:
            smallp = bctx.enter_context(tc.tile_pool(name="bsmall", bufs=3))
            psPV = bctx.enter_context(
                tc.tile_pool(name="bpspv", bufs=2, space="PSUM"))
            psCS = bctx.enter_context(
                tc.tile_pool(name="bpscs", bufs=1, space="PSUM"))
            psRB = bctx.enter_context(
                tc.tile_pool(name="bpsrb", bufs=1, space="PSUM"))
            sstack = ExitStack()
            expp = sstack.enter_context(tc.tile_pool(name="bexp", bufs=7))
            psS = sstack.enter_context(
                tc.tile_pool(name="bpss", bufs=3, space="PSUM"))

            def emit_recip(pend):
                # DVE part only — emitted early so the reciprocal clears
                # the DVE queue before the PE reaches the rbc matmul
                h, qt, pvps, csps = pend
                rec = smallp.tile([1, QT], F32R, tag="rec", name="rec")
                with nc.allow_low_precision(
                        reason="softmax denom reciprocal to f32r"):
                    nc.vector.reciprocal(rec[:], csps[:])
                return rec

            def emit_norm_rest(pend, rec):
                h, qt, pvps, csps = pend
                # broadcast SA/den to 128 partitions via K=1 PE matmul
                rbc = psRB.tile([128, QT], F32, tag="rbc", name="rbc")
                nc.tensor.matmul(rbc[:], onesr_sb[:], rec[:],
                                 start=True, stop=True)
                # ACT is saturated by exp in phase B — keep these copies
                # on DVE/Pool instead
                at_t = smallp.tile([128, QT], F32, tag="att", name="att")
                nc.vector.tensor_copy(at_t[:], pvps[:])
                tmp = smallp.tile([128, QT], F32, tag="tmp", name="tmp")
                nc.vector.tensor_tensor(tmp[:], at_t[:], rbc[:],
                                        mybir.AluOpType.mult)
                with nc.allow_low_precision(reason="fp8 attn split"):
                    nc.vector.tensor_copy(at_hi[qt][:, h, :], tmp[:])
                    nc.vector.tensor_tensor(
                        at_lo[qt][:, h, :], tmp[:],
                        at_hi[qt][:, h, :], mybir.AluOpType.subtract)

            from collections import deque
            cspv_q = deque()     # (emit_fn, key_or_None_if_not_last)
            norms = []           # dicts: pend, rec, recip_at, rest_at
            popc = [0]
            LAG = 4
            TOTAL_POPS = NH * sum((QT // 128) * (qt + 1) for qt in range(NQT))

            def pop_one():
                fn, last_key = cspv_q.popleft()
                fn()
                popc[0] += 1
                if last_key is not None:
                    norms.append({"pend": last_key,
                                  "recip_at": popc[0] + 1,
                                  "rest_at": popc[0] + 3, "rec": None})
                for nrm in norms:
                    if nrm["rec"] is None and popc[0] >= nrm["recip_at"]:
                        nrm["rec"] = emit_recip(nrm["pend"])
                for nrm in norms[:]:
                    # rests landing at the tail would serialize against the
                    # drain; defer them past phase C's first chunks instead
                    if (nrm["rec"] is not None and popc[0] >= nrm["rest_at"]
                            and nrm["rest_at"] <= TOTAL_POPS - 2):
                        emit_norm_rest(nrm["pend"], nrm["rec"])
                        norms.remove(nrm)

            for h in range(NH):
                for qt in range(NQT):
                    nallow = (QT // 128) * qt + (QT // 128)
                    pvps = psPV.tile([128, QT], F32, tag="pv", name="pv")
                    csps = psCS.tile([1, QT], F32, tag="cs", name="cs")

                    def mk_cspv(i, kc, lo, eb, pvps=pvps, csps=csps,
                                h=h, nallow=nallow):
                        def emit():
                            nc.tensor.matmul(
                                csps[:, lo:QT], ones_sb[:], eb[:, lo:QT],
                                start=(i == 0), stop=(i == nallow - 1),
                                skip_group_check=True)
                            nc.tensor.matmul(
                                pvps[:, lo:QT],
                                v_sb[kc // 4][:, kc % 4,
                                              h * D:(h + 1) * D],
                                eb[:, lo:QT],
                                start=(i == 0), stop=(i == nallow - 1),
                                skip_group_check=True)
                        return emit

                    for kc in range(nallow):
                        j = kc - (QT // 128) * qt
                        lo = max(0, 128 * j)
                        sps = psS.tile([128, QT], F32, tag="s", name="s")
                        nc.tensor.matmul(
                            sps[:, lo:QT],
                            ksb[kc // 4][:, h, (kc % 4) * 128:
                                         (kc % 4 + 1) * 128],
                            qsb[qt][:, h, lo:QT],
                            start=True, stop=True)
                        eb = expp.tile([128, QT], BF16, tag="e", name="e")
                        nc.scalar.activation(
                            eb[:, lo:QT], sps[:, lo:QT],
                            mybir.ActivationFunctionType.Exp, scale=DSCALE)
                        if j >= 0:
                            # mask on Pool: DVE is in-order and the
                            # normalize recip/tmp would delay it
                            with nc.allow_low_precision(reason="bf16 mask"):
                                nc.gpsimd.tensor_tensor(
                                    eb[:, lo:lo + 128],
                                    eb[:, lo:lo + 128].bitcast(BF16),
                                    tri_sb[:], mybir.AluOpType.mult)
                        cspv_q.append(
                            (mk_cspv(kc, kc, lo, eb),
                             (h, qt, pvps, csps) if kc == nallow - 1
                             else None))
                        if len(cspv_q) > LAG:
                            pop_one()
            while cspv_q:
                pop_one()
            for nrm in norms:
                if nrm["rec"] is None:
                    nrm["rec"] = emit_recip(nrm["pend"])

            # ============ Phase C: output projection ============
            # psS's banks are free (closed); psO shares the PSUM budget
            # with the still-live psPV/psCS/psRB so phase C's first chunks
            # can interleave with the deferred tail normalizes.
            sstack.close()
            outp = bctx.enter_context(tc.tile_pool(name="cout", bufs=2))
            psO = bctx.enter_context(
                tc.tile_pool(name="cpso", bufs=4, space="PSUM"))

            def emit_c_chunk(sc):
                ssl = bass.ts(sc, 128)
                ot = outp.tile([128, HID], BF16, tag="ot", name="ot")
                for nt in range(HID // QT):
                    nsl = bass.ts(nt, QT)
                    ps = psO.tile([128, QT], F32, tag="o", name="o")
                    ati = sc // 4
                    asl = slice((sc % 4) * 128, (sc % 4 + 1) * 128)
                    for gp in range(NH // 2):
                        g2 = slice(2 * gp, 2 * gp + 2)
                        nc.tensor.matmul(
                            ps[:], at_hi[ati][:, g2, asl], wo_hi[:, g2, nsl],
                            start=(gp == 0), stop=False, perf_mode=DR,
                            skip_group_check=True)
                    for gp in range(NH // 2):
                        g2 = slice(2 * gp, 2 * gp + 2)
                        nc.tensor.matmul(
                            ps[:], at_lo[ati][:, g2, asl], wo_hi[:, g2, nsl],
                            start=False, stop=False, perf_mode=DR,
                            skip_group_check=True)
                    for gp in range(NH // 2):
                        g2 = slice(2 * gp, 2 * gp + 2)
                        nc.tensor.matmul(
                            ps[:], at_hi[ati][:, g2, asl], wo_lo[:, g2, nsl],
                            start=False, stop=(gp == NH // 2 - 1),
                            perf_mode=DR, skip_group_check=True)
                    with nc.allow_low_precision(reason="bf16 out"):
                        if nt % 2 == 0:
                            nc.vector.tensor_scalar(
                                ot[:, nsl], ps[:], 1.0 / (SA * SW), None,
                                mybir.AluOpType.mult)
                        else:
                            nc.scalar.mul(ot[:, nsl], ps[:], 1.0 / (SA * SW))
                    if sc == S // 128 - 1:
                        # alternate hwdge queues so the tail dispatches overlap
                        eng = nc.sync if nt % 2 == 0 else nc.scalar
                        eng.dma_start(out.ap()[ssl, nsl], ot[:, nsl])
                    elif nt == 1:
                        nc.sync.dma_start(out.ap()[ssl, 0:2 * QT],
                                          ot[:, 0:2 * QT])
                if sc != S // 128 - 1:
                    nc.sync.dma_start(out.ap()[ssl, 2 * QT:HID],
                                      ot[:, 2 * QT:HID])

            emit_c_chunk(0)
            emit_c_chunk(1)
            for nrm in norms[:]:
                emit_norm_rest(nrm["pend"], nrm["rec"])
                norms.remove(nrm)
            for sc in range(2, S // 128):
                emit_c_chunk(sc)

    nc.compile()
    return nc


def _split8(x, scale):
    xs = x * scale
    hi = np.asarray(xs, dtype=NF8)
    lo = np.asarray(xs - hi.astype(np.float32), dtype=NF8)
    return hi, lo


def _prep_in_maps(hidden_states, cos, sin, w_qkv, w_o):
    hs = np.asarray(hidden_states, dtype=np.float32)
    cos = np.asarray(cos, dtype=np.float32)
    sin = np.asarray(sin, dtype=np.float32)
    w_qkv = np.asarray(w_qkv, dtype=np.float32)
    w_o = np.asarray(w_o, dtype=np.float32)

    wT = np.ascontiguousarray(w_qkv.T)          # (HID, 3*H*D)
    woTf = np.ascontiguousarray(w_o.T)          # (H*D, HID)
    cosT = np.asarray(np.ascontiguousarray(cos.T) / (SH * SW), dtype=NBF)
    sinT = np.ascontiguousarray(sin.T) / (SH * SW)
    sinS = sinT.copy()
    sinS[:64] = -sinT[:64]
    sinS = np.asarray(sinS, dtype=NBF)
    tri = np.asarray(np.triu(np.ones((128, 128), np.float32)), dtype=NBF)
    ones = np.asarray(np.ones((128, 1), np.float32), dtype=NBF)
    onesr = np.full((1, 128), SA, np.float32)

    hT = [np.ascontiguousarray(hs[b].T) for b in range(B)]
    hT8 = [_split8(h, SH) for h in hT]

    in_maps = []
    for c in range(NCORES):
        b, hg = c // 4, c % 4
        lo, hi = hg * NH * D, (hg + 1) * NH * D
        wq_hi, wq_lo = _split8(np.ascontiguousarray(wT[:, lo:hi]), SW)
        wk_hi, wk_lo = _split8(
            np.ascontiguousarray(wT[:, H * D + lo:H * D + hi]), SW)
        wv_hi, wv_lo = _split8(
            np.ascontiguousarray(wT[:, 2 * H * D + lo:2 * H * D + hi]), SW)
        wo_hi, wo_lo = _split8(np.ascontiguousarray(woTf[lo:hi, :]), SW)
        in_maps.append({
            "hT_hi": hT8[b][0], "hT_lo": hT8[b][1],
            "wq_hi": wq_hi, "wq_lo": wq_lo,
            "wk_hi": wk_hi, "wk_lo": wk_lo,
            "wv_hi": wv_hi, "wv_lo": wv_lo,
            "wo_hi": wo_hi, "wo_lo": wo_lo,
            "cosT": cosT, "sinS": sinS,
            "tri": tri, "ones": ones, "onesr": onesr,
        })
    return in_maps


def kernel(hidden_states, cos, sin, w_qkv, w_o, _trace=False):
    if "nc" not in _CACHED:
        _CACHED["nc"] = _build_nc()
    nc = _CACHED["nc"]
    in_maps = _prep_in_maps(hidden_states, cos, sin, w_qkv, w_o)
    res = run_bass_kernel_spmd(nc, in_maps, core_ids=list(range(NCORES)),
                               trace=_trace)
    _CACHED["last_result"] = res
    out = np.zeros((B, S, HID), np.float32)
    for c in range(NCORES):
        out[c // 4] += res.results[c]["out"].astype(np.float32)
    return out
